# revision 21
# baseline (speedup 1.0000x reference)
"""Per-sample dynamic-filter Conv2D (VALID, stride 1) on 8 Trainium2 NeuronCores.

Problem: X [16,128,128,128] (NHWC) conv with per-sample filters
kernel [16,3,3,128,128] (HWIO) -> out [16,126,126,128].

Sharding: pure data parallel - 2 samples per core, no communication.

Design (v9): all layout transforms on the host; device does only conv
matmuls plus PSUM evacuation. Valid-only tiling: 4 output rows per tile
(504 = 4x126 matmul columns via a 3-dim rhs access pattern that skips the
2 ow>=OW columns), so no wasted tap matmul columns:
  - Host pre-transposes X to X^T [Cin, H*W] (bf16, zero-padded past HW) and
    pre-flattens the filter to [Cin, tap*Cout] (bf16).
  - Device, per output tile (4 rows x 126 cols): 9 PSUM-accumulated bf16
    matmuls (lhsT = filter tap [ci, co], rhs = X^T window [ci, 4, 126]),
    then a DVE copy PSUM->SBUF with f32->bf16 cast, and a store DMA to DRAM
    out^T [co, p] issued on the Sync HWDGE ring (the last sample's final
    1-row tiles store via the idle ACT ring for a short final drain).
  - Head: 11 warmup matmuls on a memset tile keep the PE HAM-busy from
    engine release until the first X^T chunk lands (~11.7us: DMA issue +
    descriptor-gen + drain + receipt floor), so the real MM stream runs at
    the warm 2.4 GHz clock from its start with zero gaps (213 ns / 504-col
    matmul = stream roofline).
  - Tail: outputs are bf16 to halve store bytes (host converts to f32).
  - Host slices off the ow >= OW columns and transposes out^T back to NHWC.
bf16 inputs with f32 PSUM accumulation give rel err ~2e-3; bf16 output adds
<=0.4% per-element rounding (gate: 2e-2).
"""

import os
import sys

_BASS_PATH = "/opt/trn_rl_repo"
if _BASS_PATH not in sys.path:
    sys.path.insert(0, _BASS_PATH)

import numpy as np

import concourse.mybir as mybir  # noqa: E402
import concourse.tile as tile  # noqa: E402
from concourse import bacc  # noqa: E402

F32 = mybir.dt.float32
BF16 = mybir.dt.bfloat16

# Full-problem constants
B, H, W, CIN, COUT, KH, KW = 16, 128, 128, 128, 128, 3, 3
N_CORES = 8
S = B // N_CORES  # samples per core
P = 128
OH, OW = H - KH + 1, W - KW + 1
HW = H * W                      # input positions (16384)
NHW = OH * W                    # full-width output positions (16128)
HALO = (KH - 1) * W + (KW - 1)  # max tap offset (258)
XT_COLS = ((HW + HALO + P - 1) // P) * P  # padded X^T columns (16640)
NTILE = 504                     # output tile columns (NHW % NTILE == 0)

ROWS = 3  # output rows per tile in valid-only mode (378-column tiles)


def _tile_sizes(n_out, n_tile, split_head, split_tail):
    """Column counts per output tile covering n_out columns."""
    assert n_out % n_tile == 0
    sizes = [n_tile] * (n_out // n_tile)
    if split_head and n_tile == 504:
        # Split the first 504 into 126+378: compute starts on a tiny first
        # chunk, and the cold-clock matmuls are short.
        sizes = [126, 378] + sizes[1:]
    if split_tail and n_tile == 504:
        # Split the final 504 into 378+126 so the last store drains fast.
        sizes = sizes[:-1] + [378, 126]
    return sizes


def build_conv_nc(n_tile=NTILE, valid=False):
    """Build the per-core Bass program. Returns compiled nc."""
    assert NHW % n_tile == 0 and n_tile <= 512
    if valid:
        n_out = OH * OW          # 15876 valid positions
    else:
        n_out = NHW
    out_f32 = os.environ.get("CONV_OUT_F32", "0") == "1"
    store_act = os.environ.get("CONV_STORE_ACT", "0") == "1"
    split_tail = os.environ.get("CONV_NOSPLIT", "0") != "1"
    odt = F32 if out_f32 else BF16

    nc = bacc.Bacc("TRN2", target_bir_lowering=False, debug=False)
    xd = nc.dram_tensor("xt", [S, CIN, XT_COLS], BF16, kind="ExternalInput").ap()
    kd = nc.dram_tensor(
        "k", [S, CIN, KH * KW * COUT], BF16, kind="ExternalInput"
    ).ap()
    od = nc.dram_tensor("o", [S, COUT, n_out], odt, kind="ExternalOutput").ap()

    # X^T DMA chunk column boundaries: a small first chunk (covers tile 0)
    # heads the Sync ring so its completion sem fires as early as the
    # descriptor-generation pipeline allows, bigger chunks after.
    ch0 = int(os.environ.get("CONV_CH0", "784"))
    ch1 = int(os.environ.get("CONV_CH1", "3424"))
    nwu = int(os.environ.get("CONV_WARMUP", "10"))
    split_head = os.environ.get("CONV_SPLITHEAD", "0") == "1"
    bounds = [0, ch0, ch1]
    rest = XT_COLS - ch1
    nrest = 5
    step = ((rest // nrest) // 16 + 1) * 16
    while bounds[-1] < XT_COLS:
        bounds.append(min(bounds[-1] + step, XT_COLS))

    store_eng = nc.scalar if store_act else nc.sync
    tail_store_eng = nc.sync if store_act else nc.scalar

    with tile.TileContext(nc) as tc:
        with (
            tc.tile_pool(name="xt", bufs=2) as xt_pool,
            tc.tile_pool(name="filt", bufs=2) as filt_pool,
            tc.tile_pool(name="wusrc", bufs=1) as wusrc_pool,
            tc.tile_pool(name="ostage", bufs=16) as ostage_pool,
            tc.tile_pool(name="acc", bufs=6, space="PSUM") as acc_pool,
            tc.tile_pool(name="wu", bufs=1, space="PSUM") as wu_pool,
        ):
            state = {}
            for s in range(S):
                filt = filt_pool.tile(
                    [P, KH * KW * COUT], BF16, tag=f"filt{s}", name=f"filt{s}"
                )
                xt = xt_pool.tile([P, XT_COLS], BF16, tag=f"xt{s}", name=f"xt{s}")
                state[s] = (filt, xt)
            filt0, xt0 = state[0]
            wsrc = None
            if nwu:
                # Warmup source is read UNINITIALIZED on purpose: the warmup
                # matmuls' PSUM output is never consumed, so garbage inputs
                # are harmless, and having no producer lets the warmup start
                # at tensor-engine release (~6.5us) instead of waiting for a
                # memset - the HAM clock gate is then reliably at 8/8 before
                # the first real matmul regardless of window phase.
                wsrc = wusrc_pool.tile([P, NTILE], BF16, tag="wsrc", name="wsrc")
                # Tiny 16-col memset on DVE: satisfies Tile's must-have-a-
                # writer rule and gives the warmup matmuls a fast dependency;
                # the remaining columns are read uninitialized (harmless).
                nc.vector.memset(wsrc[:, 0:16], 1.0)
            # ALL input loads ride the gpsimd SWDGE queue, in gate order:
            # chunk0 first (its completion sem gates the first real matmul),
            # then filt0, then growing chunks. SWDGE is a separate descriptor
            # path with ~2us fixed latency, strict FIFO per queue, and it
            # keeps the Sync HWDGE ring free for output stores (whose issues
            # then start right after the first copy instead of queueing
            # behind 16 input dma_starts).
            load_eng = nc.sync
            if os.environ.get("CONV_LOAD_SWDGE", "0") == "1":
                load_eng = nc.gpsimd
            load_eng.dma_start(
                out=xt0[:, 0 : bounds[1]], in_=xd[0, :, 0 : bounds[1]]
            )
            load_eng.dma_start(out=filt0[:], in_=kd[0])
            # PE warm-up: matmuls on the memset tile (no DMA dependency)
            # keep the PE HAM-busy from the end of the preamble until the
            # first chunk lands, so the real stream runs at the warm
            # 2.4 GHz clock from (almost) its start.
            if nwu:
                wu = wu_pool.tile([P, NTILE], F32, tag="wu", name="wu")
                for _ in range(nwu):
                    nc.tensor.matmul(
                        wu[:],
                        wsrc[:, 0:COUT],
                        wsrc[:],
                        start=True,
                        stop=True,
                    )
            for s in range(S):
                filt, xt = state[s]
                for c in range(len(bounds) - 1):
                    if s == 0 and c == 0:
                        continue
                    load_eng.dma_start(
                        out=xt[:, bounds[c] : bounds[c + 1]],
                        in_=xd[s, :, bounds[c] : bounds[c + 1]],
                    )
                if s > 0:
                    load_eng.dma_start(out=filt[:], in_=kd[s])

            for s in range(S):
                filt, xt = state[s]
                if valid:
                    # `rows` output rows per tile (rows*OW cols <= 504); the
                    # last sample ends with 1-row tiles for a fast final
                    # drain.
                    rows = int(os.environ.get("CONV_ROWS", "4"))
                    full, rem = divmod(OH, rows)
                    row_counts = [rows] * full + ([rem] if rem else [])
                    if split_tail and s == S - 1:
                        if rem >= 2:
                            row_counts = row_counts[:-1] + [rem - 1, 1]
                        elif rem == 0:
                            row_counts = row_counts[:-1] + [rows - 1, 1]
                    r0 = 0
                    for rcnt in row_counts:
                        base = r0 * OW
                        cols = rcnt * OW
                        acc = acc_pool.tile([P, 504], F32, tag="acc", name="acc")
                        acc_mm = acc[:, :cols].rearrange(
                            "p (r w) -> p r w", r=rcnt
                        )
                        for tap in range(KH * KW):
                            dy, dx = divmod(tap, KW)
                            b = (r0 + dy) * W + dx
                            rhs = xt[:, b : b + rcnt * W].rearrange(
                                "ci (r w) -> ci r w", r=rcnt
                            )[:, :, :OW]
                            nc.tensor.matmul(
                                acc_mm,
                                filt[:, tap * COUT : (tap + 1) * COUT],
                                rhs,
                                start=(tap == 0),
                                stop=(tap == KH * KW - 1),
                            )
                        o = ostage_pool.tile([P, 504], odt, tag="o", name="o")
                        nc.vector.tensor_copy(o[:, :cols], acc[:, :cols])
                        eng = store_eng
                        if s == S - 1 and r0 + rcnt >= OH - 3:
                            eng = tail_store_eng
                        eng.dma_start(
                            out=od[s, :, base : base + cols], in_=o[:, :cols]
                        )
                        r0 += rcnt
                else:
                    sizes = _tile_sizes(
                        n_out,
                        n_tile,
                        split_head and s == 0,
                        split_tail and s == S - 1,
                    )
                    base = 0
                    for ti, cols in enumerate(sizes):
                        acc = acc_pool.tile([P, 504], F32, tag="acc", name="acc")
                        for tap in range(KH * KW):
                            dy, dx = divmod(tap, KW)
                            off = base + dy * W + dx
                            nc.tensor.matmul(
                                acc[:, :cols],
                                filt[:, tap * COUT : (tap + 1) * COUT],
                                xt[:, off : off + cols],
                                start=(tap == 0),
                                stop=(tap == KH * KW - 1),
                            )
                        o = ostage_pool.tile([P, 504], odt, tag="o", name="o")
                        nc.vector.tensor_copy(o[:, :cols], acc[:, :cols])
                        # The last sample's final two stores go on the
                        # other ring, which is idle by then, so the final
                        # drain chain is as short as possible.
                        eng = store_eng
                        if s == S - 1 and ti >= len(sizes) - 2:
                            eng = tail_store_eng
                        eng.dma_start(
                            out=od[s, :, base : base + cols], in_=o[:, :cols]
                        )
                        base += cols

    nc.compile()
    return nc


_NC_CACHE = {}


def _valid_mode():
    return os.environ.get("CONV_VALID", "1") == "1"


def _get_nc():
    n_tile = int(os.environ.get("CONV_NTILE", str(NTILE)))
    key = (n_tile, _valid_mode())
    if key not in _NC_CACHE:
        _NC_CACHE[key] = build_conv_nc(n_tile=n_tile, valid=key[1])
    return _NC_CACHE[key]


def make_in_maps(X, K):
    """Host-side prep: X^T (padded, bf16) + flattened filters (bf16)."""
    import ml_dtypes

    bf = ml_dtypes.bfloat16
    X = np.asarray(X, dtype=np.float32)
    K = np.asarray(K, dtype=np.float32)
    assert X.shape == (B, H, W, CIN), X.shape
    assert K.shape == (B, KH, KW, CIN, COUT), K.shape
    Xt = np.zeros((B, CIN, XT_COLS), dtype=bf)
    Xt[:, :, :HW] = X.reshape(B, HW, CIN).transpose(0, 2, 1).astype(bf)
    # [B, kh, kw, ci, co] -> [B, ci, kh*kw*co]
    Kt = np.ascontiguousarray(
        K.transpose(0, 3, 1, 2, 4).reshape(B, CIN, KH * KW * COUT).astype(bf)
    )
    return [
        {"xt": Xt[i * S : (i + 1) * S], "k": Kt[i * S : (i + 1) * S]}
        for i in range(N_CORES)
    ]


def unpack_output(results):
    """[S, COUT, n_out] per core -> full [B, OH, OW, COUT] f32."""
    if _valid_mode():
        out_t = np.empty((B, COUT, OH * OW), dtype=np.float32)
        for i in range(N_CORES):
            out_t[i * S : (i + 1) * S] = results[i]["o"].astype(np.float32)
        return np.ascontiguousarray(
            out_t.reshape(B, COUT, OH, OW).transpose(0, 2, 3, 1)
        )
    out_t = np.empty((B, COUT, NHW), dtype=np.float32)
    for i in range(N_CORES):
        out_t[i * S : (i + 1) * S] = results[i]["o"].astype(np.float32)
    # [B, co, oh, W] -> drop ow >= OW -> [B, oh, ow, co]
    return np.ascontiguousarray(
        out_t.reshape(B, COUT, OH, W)[:, :, :, :OW].transpose(0, 2, 3, 1)
    )


def kernel(**inputs):
    from concourse.bass_utils import run_bass_kernel_spmd

    nc = _get_nc()
    in_maps = make_in_maps(inputs["X"], inputs["kernel"])
    res = run_bass_kernel_spmd(nc, in_maps, list(range(N_CORES)))
    return unpack_output(res.results)
